# revision 1
# baseline (speedup 1.0000x reference)
"""Trainium2 Bass kernel for nn_Decoder_46042049413334.

Buggy 2-layer LSTM decoder with attention (B=32, T=64, S=128, D=512).

Structure (per core, batch sharded 8 ways, BS=4 examples/core):
  Phase A: xpart0 = [emb(tokens), 1] @ [W_ih0.T; b0]   (batched over all t)
  Pass  B: layer-0 recurrence over t (only W_hh0 streamed per step)
  Phase C: h2_0 = sigma_o * tanh(c2) batched; xpart1 = [h2_0, 1] @ [W_ih1.T; b1]
  Pass  D: layer-1 recurrence over t (only W_hh1 streamed per step)
  Phase E: s = sigma_o1 * tanh(c2_1); attention + out-projection (batched)

Weight gate-columns are permuted so each 512-wide N-block of the gates psum is
a complete {i,f,o,g} set for a 128-wide d-block (sub-order i,f,o,g), letting
the nonlinearity pipeline against the PE weight stream. Matmuls run in bf16
(PSUM accumulation fp32); the per-step critical chain is sigma(i,f) ->
tanh(g) -> c2 -> transpose -> bf16 copy, with sigma(o) deferred.

Row ordering is b-major everywhere: row r = b_local*T + t.
"""
import numpy as np
import ml_dtypes
from contextlib import ExitStack

import concourse.bass as bass
import concourse.bacc as bacc
import concourse.tile as tile
from concourse import mybir, masks
from concourse.bass_utils import run_bass_kernel_spmd

F32 = mybir.dt.float32
BF16 = mybir.dt.bfloat16
AF = mybir.ActivationFunctionType
NPBF = ml_dtypes.bfloat16

B, T, S, D, L, V = 32, 64, 128, 512, 2, 32000
G = 4 * D        # 2048
DS = 2 * D       # 1024
NCORES = 8
BS = B // NCORES  # 4
R = BS * T        # 256 rows per core


# ---------------------------------------------------------------- host side

def _gate_perm():
    perm = np.zeros(G, dtype=np.int64)
    base = {0: 0, 1: 512, 2: 1536, 3: 1024}  # i, f, o, g
    for j in range(G):
        nb, pos = divmod(j, 512)
        sub, dd = divmod(pos, 128)
        perm[j] = base[sub] + nb * 128 + dd
    return perm


def host_prep(inputs):
    """Build the 8 per-core input maps (layout/gather work only)."""
    perm = _gate_perm()
    tokens = np.asarray(inputs["prev_tgt_tokens"])
    embed = np.asarray(inputs["embed"], dtype=np.float32)
    enc = np.asarray(inputs["encoder_out"], dtype=np.float32)
    mask = np.asarray(inputs["src_mask"])
    hid = np.asarray(inputs["hiddens"], dtype=np.float32)
    cells = np.asarray(inputs["cells"], dtype=np.float32)
    W_ih = np.asarray(inputs["W_ih"], dtype=np.float32)
    W_hh = np.asarray(inputs["W_hh"], dtype=np.float32)
    b_ih = np.asarray(inputs["b_ih"], dtype=np.float32)
    b_hh = np.asarray(inputs["b_hh"], dtype=np.float32)
    W_in = np.asarray(inputs["W_in"], dtype=np.float32)
    b_in = np.asarray(inputs["b_in"], dtype=np.float32)
    W_out = np.asarray(inputs["W_out"], dtype=np.float32)
    b_out = np.asarray(inputs["b_out"], dtype=np.float32)

    def bf(x):
        return np.ascontiguousarray(x, dtype=NPBF)

    WIH = []
    WHH = []
    for l in range(L):
        wihT = W_ih[l].T[:, perm]
        biasrow = (b_ih[l] + b_hh[l])[perm][None, :]
        WIH.append(bf(np.concatenate([wihT, biasrow], 0)))   # [513, 2048]
        WHH.append(bf(W_hh[l].T[:, perm]))                   # [512, 2048]
    WINT = bf(W_in.T)                                        # [512, 1024]
    WOUTT = bf(np.concatenate([W_out.T, b_out[None, :]], 0))  # [1537, 512]

    in_maps = []
    for core in range(NCORES):
        bsl = slice(core * BS, (core + 1) * BS)
        xe = embed[tokens[bsl]]                              # [BS, T, D]
        Xaug = np.concatenate(
            [xe.reshape(R, D), np.ones((R, 1), np.float32)], axis=1)
        XT0 = bf(Xaug.T)                                     # [513, 256]
        enc_c = np.ascontiguousarray(enc[bsl])               # [BS, 128, 1024]
        encT_c = np.swapaxes(enc_c, 1, 2)                    # [BS, 1024, 128]
        offs = np.einsum("bsd,d->bs", enc_c, b_in) + np.where(mask[bsl], -1e9, 0.0)
        offs_rep = np.ascontiguousarray(
            np.broadcast_to(offs[:, None, :], (BS, T, S)), dtype=np.float32)
        hidT = np.swapaxes(hid[:, bsl], 1, 2)                # [L, D, BS]
        # pair layout for initial c2T: [L, pair, p, 36] cols {0:4, 32:36}
        h5 = hidT.reshape(L, 2, 2, 128, BS).transpose(0, 1, 3, 2, 4)
        hidTp = np.zeros((L, 2, 128, 36), np.float32)
        hidTp[..., 0:BS] = h5[:, :, :, 0, :]
        hidTp[..., 32:32 + BS] = h5[:, :, :, 1, :]
        in_maps.append({
            "xt0": XT0,
            "wih0": WIH[0], "whh0": WHH[0],
            "wih1": WIH[1], "whh1": WHH[1],
            "wint": WINT, "woutt": WOUTT,
            "enc": bf(enc_c), "enct": bf(encT_c), "offs": offs_rep,
            "hidt": bf(hidTp), "cells": np.ascontiguousarray(cells[:, bsl]),
            "ones1": np.ones((1, R), NPBF),
            "id4": np.eye(BS, dtype=NPBF),
        })
    return in_maps


# ------------------------------------------------------------- device build

def build_program():
    nc = bacc.Bacc("TRN2", target_bir_lowering=False, debug=False)

    XT0 = nc.dram_tensor("xt0", [513, R], BF16, kind="ExternalInput")
    WIH0 = nc.dram_tensor("wih0", [513, G], BF16, kind="ExternalInput")
    WHH0 = nc.dram_tensor("whh0", [D, G], BF16, kind="ExternalInput")
    WIH1 = nc.dram_tensor("wih1", [513, G], BF16, kind="ExternalInput")
    WHH1 = nc.dram_tensor("whh1", [D, G], BF16, kind="ExternalInput")
    WINT = nc.dram_tensor("wint", [D, DS], BF16, kind="ExternalInput")
    WOUTT = nc.dram_tensor("woutt", [DS + D + 1, D], BF16, kind="ExternalInput")
    ENC = nc.dram_tensor("enc", [BS, S, DS], BF16, kind="ExternalInput")
    ENCT = nc.dram_tensor("enct", [BS, DS, S], BF16, kind="ExternalInput")
    OFFS = nc.dram_tensor("offs", [BS, T, S], F32, kind="ExternalInput")
    HIDT = nc.dram_tensor("hidt", [L, 2, 128, 36], BF16, kind="ExternalInput")
    CELLS = nc.dram_tensor("cells", [L, BS, D], F32, kind="ExternalInput")
    ONES1 = nc.dram_tensor("ones1", [1, R], BF16, kind="ExternalInput")
    ID4 = nc.dram_tensor("id4", [BS, BS], BF16, kind="ExternalInput")
    OUT = nc.dram_tensor("out", [BS, T, D], F32, kind="ExternalOutput")

    XP0 = nc.dram_tensor("xp0", [BS, T, G], BF16, kind="Internal")
    XP1 = nc.dram_tensor("xp1", [BS, T, G], BF16, kind="Internal")
    H2S = nc.dram_tensor("h2s", [L, BS, T, D], F32, kind="Internal")

    with tile.TileContext(nc) as tc, ExitStack() as ctx:
        cpool = ctx.enter_context(tc.tile_pool(name="const", bufs=1))
        ident = cpool.tile([128, 128], F32)
        masks.make_identity(nc, ident[:])
        ones = cpool.tile([1, R], BF16)
        nc.sync.dma_start(ones[:], ONES1.ap())
        i4r = cpool.tile([BS, BS], BF16)
        nc.sync.dma_start(i4r[:], ID4.ap())

        psp = ctx.enter_context(tc.tile_pool(name="ps", bufs=1, space="PSUM"))

        def gtile(idx, shape):
            return psp.tile(shape, F32, tag=f"g{idx}", name=f"g{idx}",
                            bufs=2 if idx < 3 else 1)

        def batched_xpart(wpool, lhs_tiles, W_dram, XP_dram):
            """xpart = lhsT.T @ W  -> XP_dram (bf16)."""
            wt = [wpool.tile([128, G], BF16, tag=f"wk{k}", name=f"wk{k}")
                  for k in range(4)]
            wt.append(wpool.tile([1, G], BF16, tag="wk4", name="wk4"))
            for k in range(4):
                nc.sync.dma_start(wt[k][:], W_dram.ap()[128 * k:128 * (k + 1), :])
            nc.sync.dma_start(wt[4][:], W_dram.ap()[512:513, :])
            xpflat = XP_dram.ap().rearrange("b t g -> (b t) g")
            for mc in range(2):
                for nb in range(4):
                    ps = gtile(nb, [128, 512])
                    for k in range(5):
                        nc.tensor.matmul(
                            ps[:],
                            lhs_tiles[k][:, 128 * mc:128 * (mc + 1)],
                            wt[k][:, 512 * nb:512 * (nb + 1)],
                            start=(k == 0), stop=(k == 4))
                    sb = wpool.tile([128, 512], BF16, tag=f"stg{nb}",
                                    name=f"stg{nb}")
                    nc.scalar.copy(sb[:], ps[:])
                    nc.sync.dma_start(
                        xpflat[128 * mc:128 * (mc + 1), 512 * nb:512 * (nb + 1)],
                        sb[:])

        # hoisted W_hh loads for both layers (overlap with phase A)
        wbpool = ctx.enter_context(tc.tile_pool(name="wb", bufs=1))
        whh_all = {}
        for l, Wd in ((0, WHH0), (1, WHH1)):
            tiles = [wbpool.tile([128, G], BF16, tag=f"whh{l}k{k}",
                                 name=f"whh{l}k{k}") for k in range(4)]
            for k in range(4):
                nc.sync.dma_start(
                    tiles[k][:], Wd.ap()[128 * k:128 * (k + 1), :])
            whh_all[l] = tiles

        # ---------------- Phase A: xpart0 ----------------
        with tc.tile_pool(name="pa", bufs=1) as pa:
            xt = [pa.tile([128, R], BF16, tag=f"xt{k}", name=f"xt{k}")
                  for k in range(4)]
            xt.append(pa.tile([1, R], BF16, tag="xt4", name="xt4"))
            for k in range(4):
                nc.sync.dma_start(xt[k][:], XT0.ap()[128 * k:128 * (k + 1), :])
            nc.sync.dma_start(xt[4][:], XT0.ap()[512:513, :])
            batched_xpart(pa, xt, WIH0, XP0)

        # ---------------- Recurrence passes ----------------
        def recur(l, WHH_dram, XP_dram):
            with tc.tile_pool(name=f"pb{l}", bufs=1) as pb, \
                 tc.tile_pool(name=f"pd{l}", bufs=3) as pd:
                whh = whh_all[l]
                cl = pb.tile([BS, D], F32, tag="cells", name="cells")
                nc.sync.dma_start(cl[:], CELLS.ap()[l])
                c2T = []
                for pair in range(2):
                    tl = pd.tile([128, 36], BF16, tag=f"c2Tp{pair}",
                                 name=f"c2Tp{pair}")
                    nc.sync.dma_start(tl[:], HIDT.ap()[l, pair])
                    c2T.append(tl)

                for t in range(T):
                    xp = pd.tile([BS, G], BF16, tag="xp", name="xp")
                    nc.sync.dma_start(xp[:], XP_dram.ap()[:, t, :])
                    c2T_new = [None] * 2
                    c2h = [None] * 2
                    gps = []
                    sgs = []
                    # per-block: xpart via K=BS identity matmul + 4 W chunks
                    for nb in range(4):
                        nsl = slice(512 * nb, 512 * (nb + 1))
                        pair, sub = divmod(nb, 2)
                        ps = gtile(nb, [BS, 512])
                        gps.append(ps)
                        nc.tensor.matmul(ps[:], i4r[:], xp[:, nsl],
                                         start=True, stop=False)
                        for k in range(4):
                            nc.tensor.matmul(
                                ps[:], c2T[k // 2][:, 32 * (k % 2):32 * (k % 2) + 4],
                                whh[k][:, nsl], start=False, stop=(k == 3))
                        # critical chain: sigma(i,f), tanh(g), c2 block
                        sg = pd.tile([BS, 256], F32, tag=f"sg{nb}",
                                     name=f"sg{nb}")
                        nc.scalar.activation(sg[:], ps[:, 0:256], AF.Sigmoid)
                        sgs.append(sg)
                        tg = pd.tile([BS, 128], F32, tag=f"tg{nb}",
                                     name=f"tg{nb}")
                        nc.scalar.activation(tg[:], ps[:, 384:512], AF.Tanh)
                        t1 = pd.tile([BS, 128], F32, tag=f"t1{nb}",
                                     name=f"t1{nb}")
                        nc.vector.tensor_mul(
                            t1[:], sg[:, 128:256],
                            cl[:, 128 * nb:128 * (nb + 1)])
                        t2 = pd.tile([BS, 128], F32, tag=f"t2{nb}",
                                     name=f"t2{nb}")
                        nc.vector.tensor_mul(t2[:], sg[:, 0:128], tg[:])
                        if sub == 0:
                            c2h[pair] = pd.tile([36, 128], F32,
                                                tag=f"c2h{pair}",
                                                name=f"c2h{pair}")
                        nc.vector.tensor_add(
                            c2h[pair][32 * sub:32 * sub + 4, :], t1[:], t2[:])
                        if sub == 1:
                            # one stacked transpose per block pair
                            tp = psp.tile([128, 36], F32,
                                          tag="tp0", name="tp0")
                            nc.tensor.transpose(
                                tp[:], c2h[pair][:], ident[0:36, 0:36])
                            nt = pd.tile([128, 36], BF16,
                                         tag=f"c2Tp{pair}", name=f"c2Tp{pair}")
                            nc.vector.tensor_copy(nt[:], tp[:])
                            c2T_new[pair] = nt
                    # deferred: h2 = sigma(o) * tanh(c2) -> H2S rows
                    h2row = pd.tile([BS, D], F32, tag="h2row", name="h2row")
                    for nb in range(4):
                        pair, sub = divmod(nb, 2)
                        so = pd.tile([BS, 128], F32, tag=f"so{nb}",
                                     name=f"so{nb}")
                        nc.scalar.activation(
                            so[:], gps[nb][:, 256:384], AF.Sigmoid)
                        tc2 = pd.tile([BS, 128], F32, tag=f"tc2{nb}",
                                      name=f"tc2{nb}")
                        nc.scalar.activation(
                            tc2[:], c2h[pair][32 * sub:32 * sub + 4, :],
                            AF.Tanh)
                        nc.vector.tensor_mul(
                            h2row[:, 128 * nb:128 * (nb + 1)], so[:], tc2[:])
                    nc.sync.dma_start(H2S.ap()[l, :, t, :], h2row[:])
                    c2T = c2T_new

        recur(0, WHH0, XP0)

        # ---------------- Phase C: h2_0 batched; xpart1 ----------------
        def rows_from_stores(pool, l, tagpfx):
            """Load 2 tiles [128, 512] f32 of h2/s rows (b-major)."""
            flat = H2S.ap()[l].rearrange("b t d -> (b t) d")
            outt = []
            for mc in range(2):
                msl = slice(128 * mc, 128 * (mc + 1))
                h2 = pool.tile([128, D], F32, tag=f"{tagpfx}h{mc}",
                               name=f"{tagpfx}h{mc}")
                nc.sync.dma_start(h2[:], flat[msl, :])
                outt.append(h2)
            return outt

        def transpose_rows(pool, rows, tagpfx):
            """rows: 2 tiles [128, 512] f32 -> 4 bf16 tiles [128, 256] (T)."""
            tT = [pool.tile([128, R], BF16, tag=f"{tagpfx}T{k}",
                            name=f"{tagpfx}T{k}") for k in range(4)]
            for mc in range(2):
                for k in range(4):
                    tp = psp.tile([128, 128], F32, tag="tp0", name="tp0")
                    nc.tensor.transpose(
                        tp[:], rows[mc][:, 128 * k:128 * (k + 1)], ident[:])
                    if k % 2 == 0:
                        nc.scalar.copy(tT[k][:, 128 * mc:128 * (mc + 1)], tp[:])
                    else:
                        nc.vector.tensor_copy(
                            tT[k][:, 128 * mc:128 * (mc + 1)], tp[:])
            return tT

        with tc.tile_pool(name="pc", bufs=1) as pc:
            h2rows = rows_from_stores(pc, 0, "h")
            h2T = transpose_rows(pc, h2rows, "h")
            lhs = h2T + [ones]
            batched_xpart(pc, lhs, WIH1, XP1)

        recur(1, WHH1, XP1)

        # ---------------- Phase E: attention + out proj ----------------
        with tc.tile_pool(name="pe", bufs=1) as pe:
            srows = rows_from_stores(pe, 1, "s")
            sT = transpose_rows(pe, srows, "s")

            wint = [pe.tile([128, DS], BF16, tag=f"wi{k}", name=f"wi{k}")
                    for k in range(4)]
            for k in range(4):
                nc.sync.dma_start(wint[k][:], WINT.ap()[128 * k:128 * (k + 1), :])
            xqT = []
            for m in range(8):
                ps = gtile(m % 4, [128, R])
                for k in range(4):
                    nc.tensor.matmul(
                        ps[:], wint[k][:, 128 * m:128 * (m + 1)], sT[k][:],
                        start=(k == 0), stop=(k == 3))
                xq = pe.tile([128, R], BF16, tag=f"xq{m}", name=f"xq{m}")
                if m % 2 == 0:
                    nc.scalar.copy(xq[:], ps[:])
                else:
                    nc.vector.tensor_copy(xq[:], ps[:])
                xqT.append(xq)

            ctxT = [pe.tile([128, R], BF16, tag=f"cx{m}", name=f"cx{m}")
                    for m in range(8)]
            for b in range(BS):
                bsl = slice(T * b, T * (b + 1))
                encb = pe.tile([S, DS], BF16, tag=f"enc{b}", name=f"enc{b}")
                nc.sync.dma_start(encb[:], ENC.ap()[b])
                enctb = [pe.tile([128, S], BF16, tag=f"ect{b}{k}",
                                 name=f"ect{b}{k}") for k in range(8)]
                for k in range(8):
                    nc.sync.dma_start(
                        enctb[k][:], ENCT.ap()[b, 128 * k:128 * (k + 1), :])
                eps = gtile(2 + (b % 2), [T, S])
                for k in range(8):
                    nc.tensor.matmul(
                        eps[:], xqT[k][:, bsl], enctb[k][:],
                        start=(k == 0), stop=(k == 7))
                offsb = pe.tile([T, S], F32, tag="offs", name="offs")
                nc.sync.dma_start(offsb[:], OFFS.ap()[b])
                esb = pe.tile([T, S], F32, tag="esb", name="esb")
                nc.vector.tensor_add(esb[:], eps[:], offsb[:])
                negmax = pe.tile([T, 1], F32, tag="negmax", name="negmax")
                nc.vector.reduce_max(
                    negmax[:], esb[:], axis=mybir.AxisListType.X, negate=True)
                expE = pe.tile([T, S], F32, tag="expE", name="expE")
                den = pe.tile([T, 1], F32, tag="den", name="den")
                nc.scalar.activation(
                    expE[:], esb[:], AF.Exp, bias=negmax[:], accum_out=den[:])
                rden = pe.tile([T, 1], F32, tag="rden", name="rden")
                nc.vector.reciprocal(rden[:], den[:])
                attn = pe.tile([T, S], F32, tag="attn", name="attn")
                nc.vector.tensor_scalar_mul(attn[:], expE[:], rden[:])
                tp = psp.tile([S, T], F32, tag="tp0", name="tp0")
                nc.tensor.transpose(tp[:], attn[:], ident[0:T, 0:T])
                atsb = pe.tile([S, T], BF16, tag="atsb", name="atsb")
                nc.vector.tensor_copy(atsb[:], tp[:])
                for m in range(8):
                    psc = gtile(m % 4, [128, T])
                    nc.tensor.matmul(
                        psc[:], encb[:, 128 * m:128 * (m + 1)], atsb[:],
                        start=True, stop=True)
                    if m % 2 == 0:
                        nc.scalar.copy(ctxT[m][:, bsl], psc[:])
                    else:
                        nc.vector.tensor_copy(ctxT[m][:, bsl], psc[:])

            wout = [pe.tile([128, D], BF16, tag=f"wo{k}", name=f"wo{k}")
                    for k in range(12)]
            for k in range(12):
                nc.sync.dma_start(wout[k][:], WOUTT.ap()[128 * k:128 * (k + 1), :])
            woutb = pe.tile([1, D], BF16, tag="wo12", name="wo12")
            nc.sync.dma_start(woutb[:], WOUTT.ap()[1536:1537, :])
            outflat = OUT.ap().rearrange("b t d -> (b t) d")
            lhs_all = ctxT + sT + [ones]
            wt_all = wout + [woutb]
            for mc in range(2):
                msl = slice(128 * mc, 128 * (mc + 1))
                ps = gtile(mc, [128, D])
                for k in range(13):
                    nc.tensor.matmul(
                        ps[:], lhs_all[k][:, msl], wt_all[k][:],
                        start=(k == 0), stop=(k == 12))
                osb = pe.tile([128, D], F32, tag=f"osb{mc}", name=f"osb{mc}")
                nc.scalar.activation(osb[:], ps[:], AF.Tanh)
                nc.sync.dma_start(outflat[msl, :], osb[:])

    nc.compile()
    return nc


def assemble(results):
    full = np.concatenate([r["out"] for r in results], axis=0)  # [B, T, D]
    outs = full.transpose(1, 0, 2)                              # [T, B, D]
    return np.ascontiguousarray(outs.reshape(-1, D).reshape(-1, T, D))


_nc_cache = None


def kernel(**inputs):
    global _nc_cache
    in_maps = host_prep(inputs)
    if _nc_cache is None:
        _nc_cache = build_program()
    res = run_bass_kernel_spmd(_nc_cache, in_maps, list(range(NCORES)))
    return assemble(res.results)

